# revision 33
# baseline (speedup 1.0000x reference)
"""Causal self-attention (B=4, T=2048, C=1024, H=16, D=64) on 8 TRN2 cores.

Sharding: core c handles (batch b = c//2, head-group g = c%2 of 8 heads).
Each core computes the qkv projection for its (batch, head-group), causal
attention for its 8 heads, and a partial output projection over its 512
channels. Host sums the two partials per batch and transposes (output is
produced as yT [C, T] on device).

All matmul operands are bf16 (accumulation in f32 PSUM); rel err ~3e-3.

Layouts (all on-chip):
  x    [1024c, 2048t]  host-transposed input, bf16, per-c chunks
  qkT  [128, 512] x (8j, 4t)  q/k channels on partitions, bf16
  v    [2048t, 8*65]   t on partitions; per head 64 v-cols + ones col (bf16)
  scg  [128j, 512i]    scores transposed per j-tile (PSUM f32)
  e    [128j, 512i]    exp'd scores, bf16
  oT   [512c, 2048t]   attention out channels on partitions, bf16
  yT   [1024c, 2048t]  output transposed (f32), host transposes back

Bias handling (no bias matmuls):
  q/k bias: added during PSUM->SBUF copy via DVE tensor_scalar_add
            (channels are on partitions; bias is a [128,1] column).
  v bias:   folded into the output-projection bias on the host
            (o_norm = o_nobias + bv exactly, since softmax rows sum to 1).
  proj bias: yT layout puts output channels on partitions; added during
            the PSUM->SBUF copy via DVE tensor_scalar_add.

Causal masking: off-diagonal j-tiles skipped by construction; diagonal
tiles are exp'd unmasked (scores bounded, no overflow in bf16) and the
masked triangle is zeroed in e via a [128,128] 0/1 bf16 multiply (DVE).
Row sums come free as PV output row 64 via the ones column of v.

Emission is a single software-pipelined stream: per attention slot
(b4, h, j-tile): score MM -> exp (ACT) -> [mask (DVE)] -> PV MM at lag 3,
with qkv/proj matmul groups pulled in as PE fillers between slots, paced
by an ACT-deficit counter so the scalar engine's exp work hides under PE.
"""
import sys
from collections import deque

import numpy as np

try:
    import concourse.bass as bass
except ImportError:
    sys.path.insert(0, "/opt/trn_rl_repo")
    import concourse.bass as bass

import ml_dtypes
import concourse.mybir as mybir
import concourse.tile as tile
from concourse import bacc
from concourse.bass_utils import run_bass_kernel_spmd

F32 = mybir.dt.float32
BF16 = mybir.dt.bfloat16
Exp = mybir.ActivationFunctionType.Exp

B, T, C = 4, 2048, 1024
H, D = 16, 64
HG = 8            # heads per group
GC = HG * D       # 512 channels per head-group
N_CORES = 8
NB = T // 512     # 4 q-blocks
LAG = 6           # PV trails score by LAG slots


def _build():
    nc = bacc.Bacc("TRN2", target_bir_lowering=False, debug=False,
                   num_devices=N_CORES)

    xt_d = nc.dram_tensor("xt", [C, T], BF16, kind="ExternalInput").ap()
    wqkv_d = nc.dram_tensor("wqkv", [C, 3 * GC], BF16, kind="ExternalInput").ap()
    wp_d = nc.dram_tensor("wp", [GC, C], BF16, kind="ExternalInput").ap()
    bqk_d = nc.dram_tensor("bqkc", [128, 8], F32, kind="ExternalInput").ap()
    bp_d = nc.dram_tensor("bpc", [128, 8], F32, kind="ExternalInput").ap()
    mask_d = nc.dram_tensor("maskkeep", [128, 128], BF16, kind="ExternalInput").ap()
    y_d = nc.dram_tensor("y", [C, T], BF16, kind="ExternalOutput").ap()

    with tile.TileContext(nc) as tc:
        with (
            tc.tile_pool(name="consts", bufs=1) as consts,
            tc.tile_pool(name="wx", bufs=1) as wxp,
            tc.tile_pool(name="qk", bufs=1) as qkp,
            tc.tile_pool(name="vp", bufs=1) as vp,
            tc.tile_pool(name="oc", bufs=1) as ocp,
            tc.tile_pool(name="wpp", bufs=1) as wpp,
            tc.tile_pool(name="ep", bufs=8) as ep,
            tc.tile_pool(name="yp", bufs=4) as yp,
            tc.tile_pool(name="rcp", bufs=2) as rcp,
            tc.tile_pool(name="bcp", bufs=2) as bcp,
            tc.tile_pool(name="gemm", bufs=2, space="PSUM") as gemmp,
            tc.tile_pool(name="scg", bufs=4, space="PSUM") as scgp,
            tc.tile_pool(name="oun", bufs=2, space="PSUM") as ounp,
        ):
            # ---------------- constants + input DMAs ----------------
            # warmup: tiny matmuls on a zeroed tile so the PE p-state ramp
            # (full clock after ~3us of activity) completes before real
            # matmuls arrive from the first DMAs
            wu_sb = consts.tile([128, 128], BF16, tag="wu")
            nc.vector.memset(wu_sb[:], 0.0)
            wu_ps = gemmp.tile([128, 512], F32, tag="acc", name="acc")
            for i in range(24):
                nc.tensor.matmul(wu_ps[0:64, 0:128], wu_sb[:, 0:64],
                                 wu_sb[:, 0:128], start=True, stop=True)

            w_sb = [wxp.tile([128, 3 * GC], BF16, tag=f"w{c}", name=f"w{c}")
                    for c in range(8)]
            w0a_sb = wxp.tile([128, 768], BF16, tag="w0a", name="w0a")
            w0b_sb = wxp.tile([128, 768], BF16, tag="w0b", name="w0b")
            x0_sb = [wxp.tile([128, 512], BF16, tag=f"x0_{c}", name=f"x0_{c}")
                     for c in range(8)]
            x1_sb = [wxp.tile([128, 3 * 512], BF16, tag=f"x1_{c}", name=f"x1_{c}")
                     for c in range(8)]
            # weights + first t-chunk of x, pairwise so qkv(0) can start
            # early; chunk 0 in two half-tiles so the very first matmul only
            # waits for a 768-column transfer
            nc.sync.dma_start(w0a_sb[:], wqkv_d[0:128, 0:768])
            nc.sync.dma_start(x0_sb[0][:], xt_d[0:128, 0:512])
            nc.sync.dma_start(w0b_sb[:], wqkv_d[0:128, 768:1536])
            for c in range(1, 8):
                nc.sync.dma_start(w_sb[c][:], wqkv_d[128 * c:128 * (c + 1), :])
                nc.sync.dma_start(x0_sb[c][:], xt_d[128 * c:128 * (c + 1), 0:512])
            bqk_t = consts.tile([128, 8], F32, tag="bqk")
            nc.sync.dma_start(bqk_t[:], bqk_d[:])
            mask_t = consts.tile([128, 128], BF16, tag="mask")
            nc.sync.dma_start(mask_t[:], mask_d[:])
            bp_t = consts.tile([128, 8], F32, tag="bp")
            nc.sync.dma_start(bp_t[:], bp_d[:])
            for c in range(8):
                nc.sync.dma_start(x1_sb[c][:], xt_d[128 * c:128 * (c + 1), 512:T])
            wp_sb = [wpp.tile([128, C], BF16, tag=f"wp{cc}", name=f"wp{cc}")
                     for cc in range(4)]
            for cc in range(4):
                nc.sync.dma_start(wp_sb[cc][:], wp_d[128 * cc:128 * (cc + 1), :])

            # persistent compute tiles
            qkT = [[qkp.tile([128, 512], BF16, tag=f"qkT{j}_{t}",
                             name=f"qkT{j}_{t}") for t in range(4)]
                   for j in range(8)]
            v_sb = [vp.tile([128, HG * (D + 1)], BF16, tag=f"v{i}", name=f"v{i}")
                    for i in range(16)]
            o_cat = [[ocp.tile([128, 512], BF16, tag=f"oc{cc}_{b4}",
                               name=f"oc{cc}_{b4}") for b4 in range(NB)]
                     for cc in range(4)]
            # ones columns of v (row-sum trick)
            for i in range(16):
                nc.vector.memset(
                    v_sb[i].rearrange("p (h e) -> p h e", e=D + 1)[:, :, D:D + 1],
                    1.0)

            # ---------------- qkv projection groups ----------------
            def xs(tcc, c):
                if tcc == 0:
                    return x0_sb[c][:]
                return x1_sb[c][:, 512 * (tcc - 1):512 * tcc]

            def xs128(tcc, c, t2):
                if tcc == 0:
                    return x0_sb[c][:, 128 * t2:128 * (t2 + 1)]
                o = 512 * (tcc - 1) + 128 * t2
                return x1_sb[c][:, o:o + 128]

            def wslice(c, kind):
                # kind 0..7: qk j-tile columns; kind 8: the v block
                if c == 0:
                    if kind < 6:
                        return w0a_sb[:, 128 * kind:128 * (kind + 1)]
                    if kind < 8:
                        return w0b_sb[:, 128 * (kind - 6):128 * (kind - 5)]
                    return w0b_sb[:, 256:768]
                if kind < 8:
                    return w_sb[c][:, 128 * kind:128 * (kind + 1)]
                return w_sb[c][:, 2 * GC:3 * GC]

            proj_ready = [0] * NB  # per b4: number of o_cat cc-chunks done
            tail_flag = [False]    # releases tail-only proj groups

            class QkvGroup:
                """qk: kind in 0..7 (j-tile); v: kind in 8..11 (t2)."""
                is_qkv = True

                def __init__(self, tcc, kind, acc_ap=None):
                    self.tcc, self.kind, self.c, self.acc = tcc, kind, 0, None
                    self.acc_ap = acc_ap

                @property
                def started(self):
                    return self.c > 0

                def eligible(self):
                    return True

                def step(self):
                    c = self.c
                    if c == 0:
                        self.acc = (self.acc_ap if self.acc_ap is not None
                                    else gemmp.tile([128, 512], F32,
                                                    tag="acc", name="acc")[:])
                    if self.kind < 8:
                        nc.tensor.matmul(
                            self.acc, wslice(c, self.kind),
                            xs(self.tcc, c), start=(c == 0), stop=(c == 7))
                    else:
                        t2 = self.kind - 8
                        nc.tensor.matmul(
                            self.acc, xs128(self.tcc, c, t2),
                            wslice(c, 8), start=(c == 0), stop=(c == 7))
                    self.c += 1
                    if self.c == 8:
                        self._fin()
                        return True
                    return False

                def _fin(self):
                    if self.kind < 8:
                        jt = self.kind
                        nc.vector.tensor_scalar_add(
                            qkT[jt][self.tcc][:], self.acc,
                            bqk_t[:, jt:jt + 1])
                    else:
                        tt = 4 * self.tcc + (self.kind - 8)
                        nc.vector.tensor_copy(
                            v_sb[tt].rearrange("p (h e) -> p h e",
                                               e=D + 1)[:, :, 0:D],
                            self.acc.rearrange("p (h e) -> p h e", e=D))

            class ProjGroup:
                """yT tile (ci, b4): 4 accumulating MMs over cc + copy + DMA.
                MM(cc) only needs heads 2cc,2cc+1 of attn(b4) — gated by
                proj_ready so proj work streams in as heads finish."""
                is_qkv = False

                def __init__(self, b4, ci, tail_only=False):
                    self.b4, self.ci, self.cc, self.acc = b4, ci, 0, None
                    self.tail_only = tail_only

                @property
                def started(self):
                    return self.cc > 0

                def eligible(self):
                    if self.tail_only and not tail_flag[0]:
                        return False
                    return self.cc < proj_ready[self.b4]

                def step(self):
                    cc = self.cc
                    if cc == 0:
                        self.acc = gemmp.tile([128, 512], F32, tag="acc",
                                              name="acc")[:]
                    nc.tensor.matmul(
                        self.acc,
                        wp_sb[cc][:, 128 * self.ci:128 * (self.ci + 1)],
                        o_cat[cc][self.b4][:], start=(cc == 0), stop=(cc == 3))
                    self.cc += 1
                    if self.cc == 4:
                        self._fin()
                        return True
                    return False

                def _fin(self):
                    ysb = yp.tile([128, 512], BF16, tag="y")
                    if self.b4 == 3:
                        # tail region: half-tile copy->DMA pipeline shortens
                        # the drain chain after the final matmul
                        for hlf in range(2):
                            sl = slice(256 * hlf, 256 * (hlf + 1))
                            nc.vector.tensor_scalar_add(
                                ysb[:, sl], self.acc[:, sl],
                                bp_t[:, self.ci:self.ci + 1])
                            nc.sync.dma_start(
                                y_d[128 * self.ci:128 * (self.ci + 1),
                                    512 * self.b4 + 256 * hlf:
                                    512 * self.b4 + 256 * (hlf + 1)],
                                ysb[:, sl])
                    else:
                        nc.vector.tensor_scalar_add(
                            ysb[:], self.acc, bp_t[:, self.ci:self.ci + 1])
                        nc.sync.dma_start(
                            y_d[128 * self.ci:128 * (self.ci + 1),
                                512 * self.b4:512 * (self.b4 + 1)], ysb[:])

            # qkv(0) upfront, c-major over pairs of groups so the PE can
            # start as soon as the first (w, x) chunk pair lands
            for p0 in range(0, 12, 6):
                accs = [None, None,
                        scgp.tile([128, 512], F32, tag="scg", name="scg")[:],
                        scgp.tile([128, 512], F32, tag="scg", name="scg")[:],
                        scgp.tile([128, 512], F32, tag="scg", name="scg")[:],
                        scgp.tile([128, 512], F32, tag="scg", name="scg")[:]]
                hexa = [QkvGroup(0, p0 + i, acc_ap=accs[i]) for i in range(6)]
                for c in range(8):
                    for g in hexa:
                        g.step()

            # ---------------- filler queue ----------------
            fillers = []
            qkv_index = {}
            for tcc in range(1, 4):
                for k in range(12):
                    g = QkvGroup(tcc, k)
                    fillers.append(g)
                    qkv_index[(tcc, k)] = g

            def pull_filler():
                # continue the oldest eligible in-flight group first, else
                # start a new one (cap in-flight at 2 so gemm-pool buffer
                # reuse never lands on a still-accumulating tile)
                def fin(g):
                    fillers.remove(g)
                    if g.is_qkv:
                        del qkv_index[(g.tcc, g.kind)]

                for g in fillers:
                    if g.started and g.eligible():
                        if g.step():
                            fin(g)
                        return True
                if sum(1 for g in fillers if g.started) < 2:
                    for g in fillers:
                        if not g.started and g.eligible():
                            if g.step():
                                fin(g)
                            return True
                return False

            def ensure_group(tcc, kind):
                """Finish a qkv group now if it hasn't been emitted yet —
                called just before the first PE instruction that needs its
                output, so filler work stays spread across the pipeline."""
                g = qkv_index.pop((tcc, kind), None)
                if g is None:
                    return
                while not g.step():
                    pass
                fillers.remove(g)

            # ---------------- attention pipeline ----------------
            slots = [(b4, h, jt)
                     for b4 in range(NB)
                     for h in range(HG)
                     for jt in range(4 * b4 + 4)]
            n = len(slots)
            e_of = {}     # slot idx -> (e tile, off)
            oun_of = {}   # (b4, h) -> o_un tile
            debt = 0.0
            cur_b4 = 0

            for k in range(n + LAG):
                if k < n:
                    b4, h, jt = slots[k]
                    cur_b4 = b4
                    ht, hr = h // 2, (h % 2) * 64
                    if b4 > 0:
                        ensure_group(b4, ht)          # q columns for this head
                    if jt >= 4:
                        ensure_group(jt // 4, 4 + ht)  # k columns for this j

                    off = max(0, 128 * jt - 512 * b4)
                    w = 512 - off
                    scg = scgp.tile([128, 512], F32, tag="scg")
                    nc.tensor.matmul(
                        scg[:, off:512],
                        qkT[4 + ht][jt // 4][hr:hr + 64,
                                             128 * (jt % 4):128 * (jt % 4 + 1)],
                        qkT[ht][b4][hr:hr + 64, off:512],
                        start=True, stop=True)
                    e_t = ep.tile([128, 512], BF16, tag="e")
                    nc.scalar.activation(e_t[:, off:512], scg[:, off:512],
                                         Exp, scale=0.125)
                    if jt >= 4 * b4:  # diagonal tile: zero masked triangle
                        # alternate engines: DVE is oversubscribed at head
                        # ends (reciprocal + normalize-mul + filler copies)
                        eng = nc.vector if jt % 2 else nc.gpsimd
                        eng.tensor_mul(e_t[:, off:off + 128],
                                       e_t[:, off:off + 128], mask_t[:])
                    e_of[k] = (e_t, off)
                    debt += w * 0.833 + 185.0 - w * 0.4167
                if k >= LAG:
                    b4p, hp, jtp = slots[k - LAG]
                    if jtp >= 4:
                        ensure_group(jtp // 4, 8 + jtp % 4)  # v tile for PV
                    e_t, off = e_of.pop(k - LAG)
                    njt = 4 * b4p + 4
                    if jtp == 0:
                        oun_of[(b4p, hp)] = ounp.tile([D + 1, 512], F32,
                                                      tag="oun", name="oun")
                    o_un = oun_of[(b4p, hp)]
                    nc.tensor.matmul(
                        o_un[:, off:512],
                        v_sb[jtp][:, (D + 1) * hp:(D + 1) * (hp + 1)],
                        e_t[:, off:512],
                        start=(jtp == 0), stop=(jtp == njt - 1))
                    debt -= (512 - off) * 0.4167
                    if jtp == njt - 1:
                        del oun_of[(b4p, hp)]
                        htp, hrp = hp // 2, (hp % 2) * 64
                        rc = rcp.tile([1, 512], F32, tag="rc")
                        nc.vector.reciprocal(rc[:], o_un[D:D + 1, :])
                        bc = bcp.tile([D, 512], F32, tag="bc")
                        nc.gpsimd.partition_broadcast(bc[:], rc[:])
                        nc.vector.tensor_mul(
                            o_cat[htp][b4p][hrp:hrp + 64, :],
                            o_un[0:D, :], bc[:])
                        if hp % 2 == 1:
                            proj_ready[b4p] += 1
                        if hp == 1:
                            for ci in range(8):
                                fillers.append(ProjGroup(b4p, ci))
                # spend PE filler work to cover the ACT deficit
                while debt > 0.0 and fillers:
                    if not pull_filler():
                        break
                    debt -= 213.0
                debt = min(debt, 4000.0)

            tail_flag[0] = True
            while fillers:
                if not pull_filler():
                    g = fillers[0]
                    while not g.step():
                        pass
                    fillers.remove(g)

    nc.compile()
    return nc


_NC = None


def _get_nc():
    global _NC
    if _NC is None:
        _NC = _build()
    return _NC


def _in_maps(x, W_qkv, b_qkv, W_proj, b_proj):
    bf = ml_dtypes.bfloat16
    x = np.asarray(x, dtype=np.float32)
    W_qkv = np.asarray(W_qkv, dtype=np.float32)
    b_qkv = np.asarray(b_qkv, dtype=np.float32)
    W_proj = np.asarray(W_proj, dtype=np.float32)
    b_proj = np.asarray(b_proj, dtype=np.float32)

    maskkeep = (np.arange(128)[None, :] >= np.arange(128)[:, None]).astype(bf)
    xts = [np.ascontiguousarray(x[b].T).astype(bf) for b in range(B)]

    per_g = []
    for g in range(2):
        qs, ks, vs = g * GC, C + g * GC, 2 * C + g * GC
        wqkv = np.ascontiguousarray(
            np.concatenate([W_qkv[:, qs:qs + GC], W_qkv[:, ks:ks + GC],
                            W_qkv[:, vs:vs + GC]], axis=1)).astype(bf)
        wp = np.ascontiguousarray(W_proj[g * GC:(g + 1) * GC, :]).astype(bf)
        bqk = np.concatenate([b_qkv[qs:qs + GC], b_qkv[ks:ks + GC]])
        bqk_cols = np.ascontiguousarray(bqk.reshape(8, 128).T)
        # v bias folded into proj bias: o_norm = o_nobias + bv exactly
        bp_eff = b_qkv[vs:vs + GC] @ W_proj[g * GC:(g + 1) * GC, :]
        if g == 0:
            bp_eff = bp_eff + b_proj
        bp_cols = np.ascontiguousarray(bp_eff.reshape(8, 128).T.astype(np.float32))
        per_g.append((wqkv, wp, bqk_cols, bp_cols))

    maps = []
    for core in range(N_CORES):
        b, g = core // 2, core % 2
        wqkv, wp, bqk_cols, bp_cols = per_g[g]
        maps.append({
            "xt": xts[b],
            "wqkv": wqkv,
            "wp": wp,
            "bqkc": bqk_cols,
            "bpc": bp_cols,
            "maskkeep": maskkeep,
        })
    return maps


def kernel(x, W_qkv, b_qkv, W_proj, b_proj, _trace=False, _trace_kwargs=None):
    nc = _get_nc()
    maps = _in_maps(x, W_qkv, b_qkv, W_proj, b_proj)
    br = run_bass_kernel_spmd(nc, maps, list(range(N_CORES)),
                              trace=_trace, **(_trace_kwargs or {}))
    out = np.empty((B, T, C), dtype=np.float32)
    for b in range(B):
        out[b] = (br.results[2 * b]["y"].astype(np.float32)
                  + br.results[2 * b + 1]["y"].astype(np.float32)).T
    kernel._last_results = br
    return out


# revision 34
# speedup vs baseline: 1.0168x; 1.0168x over previous
"""Causal self-attention (B=4, T=2048, C=1024, H=16, D=64) on 8 TRN2 cores.

Sharding: core c handles (batch b = c//2, head-group g = c%2 of 8 heads).
Each core computes the qkv projection for its (batch, head-group), causal
attention for its 8 heads, and a partial output projection over its 512
channels. Host sums the two partials per batch and transposes (output is
produced as yT [C, T] on device).

All matmul operands are bf16 (accumulation in f32 PSUM); rel err ~3e-3.

Layouts (all on-chip):
  x    [1024c, 2048t]  host-transposed input, bf16, per-c chunks
  qkT  [128, 512] x (8j, 4t)  q/k channels on partitions, bf16
  v    [2048t, 8*65]   t on partitions; per head 64 v-cols + ones col (bf16)
  scg  [128j, 512i]    scores transposed per j-tile (PSUM f32)
  e    [128j, 512i]    exp'd scores, bf16
  oT   [512c, 2048t]   attention out channels on partitions, bf16
  yT   [1024c, 2048t]  output transposed (f32), host transposes back

Bias handling (no bias matmuls):
  q/k bias: added during PSUM->SBUF copy via DVE tensor_scalar_add
            (channels are on partitions; bias is a [128,1] column).
  v bias:   folded into the output-projection bias on the host
            (o_norm = o_nobias + bv exactly, since softmax rows sum to 1).
  proj bias: yT layout puts output channels on partitions; added during
            the PSUM->SBUF copy via DVE tensor_scalar_add.

Causal masking: off-diagonal j-tiles skipped by construction; diagonal
tiles are exp'd unmasked (scores bounded, no overflow in bf16) and the
masked triangle is zeroed in e via a [128,128] 0/1 bf16 multiply (DVE).
Row sums come free as PV output row 64 via the ones column of v.

Emission is a single software-pipelined stream: per attention slot
(b4, h, j-tile): score MM -> exp (ACT) -> [mask (DVE)] -> PV MM at lag 3,
with qkv/proj matmul groups pulled in as PE fillers between slots, paced
by an ACT-deficit counter so the scalar engine's exp work hides under PE.
"""
import sys
from collections import deque

import numpy as np

try:
    import concourse.bass as bass
except ImportError:
    sys.path.insert(0, "/opt/trn_rl_repo")
    import concourse.bass as bass

import ml_dtypes
import concourse.mybir as mybir
import concourse.tile as tile
from concourse import bacc
from concourse.bass_utils import run_bass_kernel_spmd

F32 = mybir.dt.float32
BF16 = mybir.dt.bfloat16
Exp = mybir.ActivationFunctionType.Exp

B, T, C = 4, 2048, 1024
H, D = 16, 64
HG = 8            # heads per group
GC = HG * D       # 512 channels per head-group
N_CORES = 8
NB = T // 512     # 4 q-blocks
LAG = 6           # PV trails score by LAG slots


def _build():
    nc = bacc.Bacc("TRN2", target_bir_lowering=False, debug=False,
                   num_devices=N_CORES)

    xt_d = nc.dram_tensor("xt", [C, T], BF16, kind="ExternalInput").ap()
    wqkv_d = nc.dram_tensor("wqkv", [C, 3 * GC], BF16, kind="ExternalInput").ap()
    wp_d = nc.dram_tensor("wp", [GC, C], BF16, kind="ExternalInput").ap()
    bqk_d = nc.dram_tensor("bqkc", [128, 8], F32, kind="ExternalInput").ap()
    bp_d = nc.dram_tensor("bpc", [128, 8], F32, kind="ExternalInput").ap()
    mask_d = nc.dram_tensor("maskkeep", [128, 128], BF16, kind="ExternalInput").ap()
    y_d = nc.dram_tensor("y", [C, T], BF16, kind="ExternalOutput").ap()

    with tile.TileContext(nc) as tc:
        with (
            tc.tile_pool(name="consts", bufs=1) as consts,
            tc.tile_pool(name="wx", bufs=1) as wxp,
            tc.tile_pool(name="qk", bufs=1) as qkp,
            tc.tile_pool(name="vp", bufs=1) as vp,
            tc.tile_pool(name="oc", bufs=1) as ocp,
            tc.tile_pool(name="wpp", bufs=1) as wpp,
            tc.tile_pool(name="ep", bufs=8) as ep,
            tc.tile_pool(name="yp", bufs=4) as yp,
            tc.tile_pool(name="rcp", bufs=2) as rcp,
            tc.tile_pool(name="bcp", bufs=2) as bcp,
            tc.tile_pool(name="gemm", bufs=2, space="PSUM") as gemmp,
            tc.tile_pool(name="scg", bufs=4, space="PSUM") as scgp,
            tc.tile_pool(name="oun", bufs=2, space="PSUM") as ounp,
        ):
            # ---------------- constants + input DMAs ----------------
            # warmup: tiny matmuls on a zeroed tile so the PE p-state ramp
            # (full clock after ~3us of activity) completes before real
            # matmuls arrive from the first DMAs
            wu_sb = consts.tile([128, 128], BF16, tag="wu")
            nc.vector.memset(wu_sb[:], 0.0)
            wu_ps = gemmp.tile([128, 512], F32, tag="acc", name="acc")
            for i in range(24):
                nc.tensor.matmul(wu_ps[0:64, 0:128], wu_sb[:, 0:64],
                                 wu_sb[:, 0:128], start=True, stop=True)

            w_sb = [wxp.tile([128, 3 * GC], BF16, tag=f"w{c}", name=f"w{c}")
                    for c in range(8)]
            w0a_sb = wxp.tile([128, 768], BF16, tag="w0a", name="w0a")
            w0b_sb = wxp.tile([128, 768], BF16, tag="w0b", name="w0b")
            x0_sb = [wxp.tile([128, 512], BF16, tag=f"x0_{c}", name=f"x0_{c}")
                     for c in range(8)]
            x1_sb = [wxp.tile([128, 3 * 512], BF16, tag=f"x1_{c}", name=f"x1_{c}")
                     for c in range(8)]
            # weights + first t-chunk of x, pairwise so qkv(0) can start
            # early; chunk 0 in two half-tiles so the very first matmul only
            # waits for a 768-column transfer
            nc.sync.dma_start(w0a_sb[:], wqkv_d[0:128, 0:768])
            nc.sync.dma_start(x0_sb[0][:], xt_d[0:128, 0:512])
            nc.sync.dma_start(w0b_sb[:], wqkv_d[0:128, 768:1536])
            for c in range(1, 8):
                nc.sync.dma_start(w_sb[c][:], wqkv_d[128 * c:128 * (c + 1), :])
                nc.sync.dma_start(x0_sb[c][:], xt_d[128 * c:128 * (c + 1), 0:512])
            bqk_t = consts.tile([128, 8], F32, tag="bqk")
            nc.sync.dma_start(bqk_t[:], bqk_d[:])
            mask_t = consts.tile([128, 128], BF16, tag="mask")
            nc.sync.dma_start(mask_t[:], mask_d[:])
            bp_t = consts.tile([128, 8], F32, tag="bp")
            nc.sync.dma_start(bp_t[:], bp_d[:])
            for c in range(8):
                nc.sync.dma_start(x1_sb[c][:], xt_d[128 * c:128 * (c + 1), 512:T])
            wp_sb = [wpp.tile([128, C], BF16, tag=f"wp{cc}", name=f"wp{cc}")
                     for cc in range(4)]
            for cc in range(4):
                nc.sync.dma_start(wp_sb[cc][:], wp_d[128 * cc:128 * (cc + 1), :])

            # persistent compute tiles
            qkT = [[qkp.tile([128, 512], BF16, tag=f"qkT{j}_{t}",
                             name=f"qkT{j}_{t}") for t in range(4)]
                   for j in range(8)]
            v_sb = [vp.tile([128, HG * (D + 1)], BF16, tag=f"v{i}", name=f"v{i}")
                    for i in range(16)]
            o_cat = [[ocp.tile([128, 512], BF16, tag=f"oc{cc}_{b4}",
                               name=f"oc{cc}_{b4}") for b4 in range(NB)]
                     for cc in range(4)]
            # ones columns of v (row-sum trick)
            for i in range(16):
                nc.vector.memset(
                    v_sb[i].rearrange("p (h e) -> p h e", e=D + 1)[:, :, D:D + 1],
                    1.0)

            # ---------------- qkv projection groups ----------------
            def xs(tcc, c):
                if tcc == 0:
                    return x0_sb[c][:]
                return x1_sb[c][:, 512 * (tcc - 1):512 * tcc]

            def xs128(tcc, c, t2):
                if tcc == 0:
                    return x0_sb[c][:, 128 * t2:128 * (t2 + 1)]
                o = 512 * (tcc - 1) + 128 * t2
                return x1_sb[c][:, o:o + 128]

            def wslice(c, kind):
                # kind 0..7: qk j-tile columns; kind 8: the v block
                if c == 0:
                    if kind < 6:
                        return w0a_sb[:, 128 * kind:128 * (kind + 1)]
                    if kind < 8:
                        return w0b_sb[:, 128 * (kind - 6):128 * (kind - 5)]
                    return w0b_sb[:, 256:768]
                if kind < 8:
                    return w_sb[c][:, 128 * kind:128 * (kind + 1)]
                return w_sb[c][:, 2 * GC:3 * GC]

            proj_ready = [0] * NB  # per b4: number of o_cat cc-chunks done
            tail_flag = [False]    # releases tail-only proj groups

            class QkvGroup:
                """qk: kind in 0..7 (j-tile); v: kind in 8..11 (t2)."""
                is_qkv = True

                def __init__(self, tcc, kind, acc_ap=None):
                    self.tcc, self.kind, self.c, self.acc = tcc, kind, 0, None
                    self.acc_ap = acc_ap

                @property
                def started(self):
                    return self.c > 0

                def eligible(self):
                    return True

                def step(self):
                    c = self.c
                    if c == 0:
                        self.acc = (self.acc_ap if self.acc_ap is not None
                                    else gemmp.tile([128, 512], F32,
                                                    tag="acc", name="acc")[:])
                    if self.kind < 8:
                        nc.tensor.matmul(
                            self.acc, wslice(c, self.kind),
                            xs(self.tcc, c), start=(c == 0), stop=(c == 7))
                    else:
                        t2 = self.kind - 8
                        nc.tensor.matmul(
                            self.acc, xs128(self.tcc, c, t2),
                            wslice(c, 8), start=(c == 0), stop=(c == 7))
                    self.c += 1
                    if self.c == 8:
                        self._fin()
                        return True
                    return False

                def _fin(self):
                    if self.kind < 8:
                        jt = self.kind
                        nc.vector.tensor_scalar_add(
                            qkT[jt][self.tcc][:], self.acc,
                            bqk_t[:, jt:jt + 1])
                    else:
                        tt = 4 * self.tcc + (self.kind - 8)
                        nc.vector.tensor_copy(
                            v_sb[tt].rearrange("p (h e) -> p h e",
                                               e=D + 1)[:, :, 0:D],
                            self.acc.rearrange("p (h e) -> p h e", e=D))

            class ProjGroup:
                """yT tile (ci, b4): 4 accumulating MMs over cc + copy + DMA.
                MM(cc) only needs heads 2cc,2cc+1 of attn(b4) — gated by
                proj_ready so proj work streams in as heads finish."""
                is_qkv = False

                def __init__(self, b4, ci, tail_only=False):
                    self.b4, self.ci, self.cc, self.acc = b4, ci, 0, None
                    self.tail_only = tail_only

                @property
                def started(self):
                    return self.cc > 0

                def eligible(self):
                    if self.tail_only and not tail_flag[0]:
                        return False
                    return self.cc < proj_ready[self.b4]

                def step(self):
                    cc = self.cc
                    if cc == 0:
                        self.acc = gemmp.tile([128, 512], F32, tag="acc",
                                              name="acc")[:]
                    nc.tensor.matmul(
                        self.acc,
                        wp_sb[cc][:, 128 * self.ci:128 * (self.ci + 1)],
                        o_cat[cc][self.b4][:], start=(cc == 0), stop=(cc == 3))
                    self.cc += 1
                    if self.cc == 4:
                        self._fin()
                        return True
                    return False

                def _fin(self):
                    ysb = yp.tile([128, 512], BF16, tag="y")
                    nc.vector.tensor_scalar_add(ysb[:], self.acc,
                                                bp_t[:, self.ci:self.ci + 1])
                    nc.sync.dma_start(
                        y_d[128 * self.ci:128 * (self.ci + 1),
                            512 * self.b4:512 * (self.b4 + 1)], ysb[:])

            # qkv(0) upfront, c-major over pairs of groups so the PE can
            # start as soon as the first (w, x) chunk pair lands
            for p0 in range(0, 12, 6):
                accs = [None, None,
                        scgp.tile([128, 512], F32, tag="scg", name="scg")[:],
                        scgp.tile([128, 512], F32, tag="scg", name="scg")[:],
                        scgp.tile([128, 512], F32, tag="scg", name="scg")[:],
                        scgp.tile([128, 512], F32, tag="scg", name="scg")[:]]
                hexa = [QkvGroup(0, p0 + i, acc_ap=accs[i]) for i in range(6)]
                for c in range(8):
                    for g in hexa:
                        g.step()

            # ---------------- filler queue ----------------
            fillers = []
            qkv_index = {}
            for tcc in range(1, 4):
                for k in range(12):
                    g = QkvGroup(tcc, k)
                    fillers.append(g)
                    qkv_index[(tcc, k)] = g

            def pull_filler():
                # continue the oldest eligible in-flight group first, else
                # start a new one (cap in-flight at 2 so gemm-pool buffer
                # reuse never lands on a still-accumulating tile)
                def fin(g):
                    fillers.remove(g)
                    if g.is_qkv:
                        del qkv_index[(g.tcc, g.kind)]

                for g in fillers:
                    if g.started and g.eligible():
                        if g.step():
                            fin(g)
                        return True
                if sum(1 for g in fillers if g.started) < 2:
                    for g in fillers:
                        if not g.started and g.eligible():
                            if g.step():
                                fin(g)
                            return True
                return False

            def ensure_group(tcc, kind):
                """Finish a qkv group now if it hasn't been emitted yet —
                called just before the first PE instruction that needs its
                output, so filler work stays spread across the pipeline."""
                g = qkv_index.pop((tcc, kind), None)
                if g is None:
                    return
                while not g.step():
                    pass
                fillers.remove(g)

            # ---------------- attention pipeline ----------------
            slots = [(b4, h, jt)
                     for b4 in range(NB)
                     for h in range(HG)
                     for jt in range(4 * b4 + 4)]
            n = len(slots)
            e_of = {}     # slot idx -> (e tile, off)
            oun_of = {}   # (b4, h) -> o_un tile
            debt = 0.0
            cur_b4 = 0

            for k in range(n + LAG):
                if k < n:
                    b4, h, jt = slots[k]
                    cur_b4 = b4
                    ht, hr = h // 2, (h % 2) * 64
                    if b4 > 0:
                        ensure_group(b4, ht)          # q columns for this head
                    if jt >= 4:
                        ensure_group(jt // 4, 4 + ht)  # k columns for this j

                    off = max(0, 128 * jt - 512 * b4)
                    w = 512 - off
                    scg = scgp.tile([128, 512], F32, tag="scg")
                    nc.tensor.matmul(
                        scg[:, off:512],
                        qkT[4 + ht][jt // 4][hr:hr + 64,
                                             128 * (jt % 4):128 * (jt % 4 + 1)],
                        qkT[ht][b4][hr:hr + 64, off:512],
                        start=True, stop=True)
                    e_t = ep.tile([128, 512], BF16, tag="e")
                    nc.scalar.activation(e_t[:, off:512], scg[:, off:512],
                                         Exp, scale=0.125)
                    if jt >= 4 * b4:  # diagonal tile: zero masked triangle
                        # alternate engines: DVE is oversubscribed at head
                        # ends (reciprocal + normalize-mul + filler copies)
                        eng = nc.vector if jt % 2 else nc.gpsimd
                        eng.tensor_mul(e_t[:, off:off + 128],
                                       e_t[:, off:off + 128], mask_t[:])
                    e_of[k] = (e_t, off)
                    debt += w * 0.833 + 185.0 - w * 0.4167
                if k >= LAG:
                    b4p, hp, jtp = slots[k - LAG]
                    if jtp >= 4:
                        ensure_group(jtp // 4, 8 + jtp % 4)  # v tile for PV
                    e_t, off = e_of.pop(k - LAG)
                    njt = 4 * b4p + 4
                    if jtp == 0:
                        oun_of[(b4p, hp)] = ounp.tile([D + 1, 512], F32,
                                                      tag="oun", name="oun")
                    o_un = oun_of[(b4p, hp)]
                    nc.tensor.matmul(
                        o_un[:, off:512],
                        v_sb[jtp][:, (D + 1) * hp:(D + 1) * (hp + 1)],
                        e_t[:, off:512],
                        start=(jtp == 0), stop=(jtp == njt - 1))
                    debt -= (512 - off) * 0.4167
                    if jtp == njt - 1:
                        del oun_of[(b4p, hp)]
                        htp, hrp = hp // 2, (hp % 2) * 64
                        rc = rcp.tile([1, 512], F32, tag="rc")
                        nc.vector.reciprocal(rc[:], o_un[D:D + 1, :])
                        bc = bcp.tile([D, 512], F32, tag="bc")
                        nc.gpsimd.partition_broadcast(bc[:], rc[:])
                        nc.vector.tensor_mul(
                            o_cat[htp][b4p][hrp:hrp + 64, :],
                            o_un[0:D, :], bc[:])
                        if hp % 2 == 1:
                            proj_ready[b4p] += 1
                        if hp == 1:
                            for ci in range(8):
                                fillers.append(ProjGroup(b4p, ci))
                # spend PE filler work to cover the ACT deficit
                while debt > 0.0 and fillers:
                    if not pull_filler():
                        break
                    debt -= 213.0
                debt = min(debt, 4000.0)

            tail_flag[0] = True
            while fillers:
                if not pull_filler():
                    g = fillers[0]
                    while not g.step():
                        pass
                    fillers.remove(g)

    nc.compile()
    return nc


_NC = None


def _get_nc():
    global _NC
    if _NC is None:
        _NC = _build()
    return _NC


def _in_maps(x, W_qkv, b_qkv, W_proj, b_proj):
    bf = ml_dtypes.bfloat16
    x = np.asarray(x, dtype=np.float32)
    W_qkv = np.asarray(W_qkv, dtype=np.float32)
    b_qkv = np.asarray(b_qkv, dtype=np.float32)
    W_proj = np.asarray(W_proj, dtype=np.float32)
    b_proj = np.asarray(b_proj, dtype=np.float32)

    maskkeep = (np.arange(128)[None, :] >= np.arange(128)[:, None]).astype(bf)
    xts = [np.ascontiguousarray(x[b].T).astype(bf) for b in range(B)]

    per_g = []
    for g in range(2):
        qs, ks, vs = g * GC, C + g * GC, 2 * C + g * GC
        wqkv = np.ascontiguousarray(
            np.concatenate([W_qkv[:, qs:qs + GC], W_qkv[:, ks:ks + GC],
                            W_qkv[:, vs:vs + GC]], axis=1)).astype(bf)
        wp = np.ascontiguousarray(W_proj[g * GC:(g + 1) * GC, :]).astype(bf)
        bqk = np.concatenate([b_qkv[qs:qs + GC], b_qkv[ks:ks + GC]])
        bqk_cols = np.ascontiguousarray(bqk.reshape(8, 128).T)
        # v bias folded into proj bias: o_norm = o_nobias + bv exactly
        bp_eff = b_qkv[vs:vs + GC] @ W_proj[g * GC:(g + 1) * GC, :]
        if g == 0:
            bp_eff = bp_eff + b_proj
        bp_cols = np.ascontiguousarray(bp_eff.reshape(8, 128).T.astype(np.float32))
        per_g.append((wqkv, wp, bqk_cols, bp_cols))

    maps = []
    for core in range(N_CORES):
        b, g = core // 2, core % 2
        wqkv, wp, bqk_cols, bp_cols = per_g[g]
        maps.append({
            "xt": xts[b],
            "wqkv": wqkv,
            "wp": wp,
            "bqkc": bqk_cols,
            "bpc": bp_cols,
            "maskkeep": maskkeep,
        })
    return maps


def kernel(x, W_qkv, b_qkv, W_proj, b_proj, _trace=False, _trace_kwargs=None):
    nc = _get_nc()
    maps = _in_maps(x, W_qkv, b_qkv, W_proj, b_proj)
    br = run_bass_kernel_spmd(nc, maps, list(range(N_CORES)),
                              trace=_trace, **(_trace_kwargs or {}))
    out = np.empty((B, T, C), dtype=np.float32)
    for b in range(B):
        out[b] = (br.results[2 * b]["y"].astype(np.float32)
                  + br.results[2 * b + 1]["y"].astype(np.float32)).T
    kernel._last_results = br
    return out
